# revision 28
# baseline (speedup 1.0000x reference)
"""Trainium2 Bass kernel for the 2-layer grid-GCN + linear head.

Math: the GCN aggregation over the fixed graph is a linear operator on
the node axis: out = A @ h per batch column, where
A[j, i] = sum_{edges (i->j)} dinv[i]*dinv[j].  For the 26x26 grid with
row-major node order A is banded (|i-j| <= 26).  The whole network is

    h1 = relu(B1 @ xT + b1)      B1 = w1 * A
    h2 = relu(B2 @ h1 + b2)      B2 = w2 * A
    y  = relu(linw.T @ h2 + lin_b)

Batch is sharded across the 8 NeuronCores (pure data parallel).

Fast path (fp8 DoubleRow): when w2 <= 0 and b2 <= 0, the network output
is EXACTLY relu(lin_b) for any x/w1/b1: h1 >= 0 (relu), A >= 0
entrywise, so w2*A@h1 + b2 <= 0 and h2 == 0 identically — in any
rounding mode that preserves sign (fp8 does).  So fp8 precision is
bit-exact on such instances while the tensor engine runs DoubleRow
matmuls (2 rhs rows/cycle, K=256 per instruction):

  conv1: 6 DoubleRow matmuls; output tiles = aligned [0,128) plus
         shifted [64+128j, 192+128j) so every tile's +-26 dependency
         band fits an aligned K=256 window of x tiles.
  conv2: 6 DoubleRow matmuls back to ALIGNED output tiles, reading
         contiguous pairs of the shifted h1 slots.
  head:  3 DoubleRow matmuls (K=256 each) accumulating one PSUM.

PSUM->SBUF relu+cast drains alternate Scalar/Vector (GPSIMD cannot
read PSUM); emission is software-pipelined conv1(c)|conv2(c-1)|head(c-2)
with conv1/conv2 matmuls interleaved for PSUM-reuse slack.

General fallback (w2 > 0 or b2 > 0): the bf16 block-tridiagonal kernel
(rel err ~1e-3), below.
"""

import sys

if "/opt/trn_rl_repo" not in sys.path:
    sys.path.insert(0, "/opt/trn_rl_repo")

import numpy as np
import ml_dtypes

N_CORES = 8
N = 676           # nodes (26x26 grid)
NP = 768          # padded to 6 x 128
B_TOTAL = 65536
COLS = B_TOTAL // N_CORES      # batch columns per core
CHUNK = 512                    # matmul free dim / PSUM bank
GROUP = 2048                   # DMA column-group
N_CHUNKS = COLS // CHUNK
N_GROUPS = COLS // GROUP
N_TILES = (N + 127) // 128     # 6 node tiles
P = [min(128, N - 128 * t) for t in range(N_TILES)]   # [128]*5 + [36]
OFF = [128 * t for t in range(N_TILES)]

bf16 = ml_dtypes.bfloat16
e4m3 = ml_dtypes.float8_e4m3

TRACE = False            # test.py flips this to profile
LAST_RESULT = None       # BassKernelResults stash when TRACE

# ---------------------------------------------------------------------------
# fp8 DoubleRow fast path
# ---------------------------------------------------------------------------

# h1 slot s covers nodes [SLOT_BASE[s], SLOT_BASE[s]+128)
SLOT_BASE = [0, 64, 192, 320, 448, 576]
# conv1 matmul s: (out_base, x-tile index k -> halves (k, k+1))
CONV1_SPECS = [(0, 0)] + [(64 + 128 * j, j) for j in range(5)]
# conv2 out tile t reads h1 slot pair (a, a+1); a per tile:
CONV2_SLOT = [0, 1, 2, 3, 4, 4]


def _pack_fp8_weights(B1, B2, lw):
    """lhsT blocks [128, 2, 128]: w[p, h, m] = B[ob+m, ib_h+p]."""

    def make_w(B, ob, ibs, t0rule):
        w = np.zeros((128, 2, 128), np.float32)
        for h, ib in enumerate(ibs):
            ipmax = min(128, N - ib)
            ommax = min(128, N - ob)
            if ipmax > 0 and ommax > 0:
                w[:ipmax, h, :ommax] = B[ob:ob + ommax, ib:ib + ipmax].T
        if t0rule:  # conv2 t=0: in-nodes [64,128) owned by half0
            w[:64, 1, :] = 0.0
        return w

    wc1 = np.zeros((128, 6, 2, 128), e4m3)
    for s, (ob, k) in enumerate(CONV1_SPECS):
        wc1[:, s] = make_w(B1, ob, (128 * k, 128 * k + 128), False).astype(e4m3)
    wc2 = np.zeros((128, 6, 2, 128), e4m3)
    for t in range(6):
        a = CONV2_SLOT[t]
        wc2[:, t] = make_w(
            B2, 128 * t, (SLOT_BASE[a], SLOT_BASE[a + 1]), t == 0
        ).astype(e4m3)
    wh = np.zeros((128, 3, 2, 16), e4m3)
    lwp = np.zeros(NP, np.float32)
    lwp[:N] = lw
    for k in range(3):
        for h in range(2):
            base = 256 * k + 128 * h
            wh[:, k, h, 0] = lwp[base:base + 128].astype(e4m3)
    return wc1, wc2, wh


_PROGRAM_CACHE = {}


def _build_fp8_program(b1f, b2f, linbf):
    key = ("fp8", b1f, b2f, linbf)
    if key in _PROGRAM_CACHE:
        return _PROGRAM_CACHE[key]

    import concourse.mybir as mybir
    import concourse.tile as tile
    from concourse import bacc

    nc = bacc.Bacc(None, target_bir_lowering=False)
    dt = mybir.dt
    DR = mybir.MatmulPerfMode.DoubleRow
    relu = mybir.ActivationFunctionType.Relu

    xt_d = nc.dram_tensor("xt", (NP, COLS), dt.float8e4, kind="ExternalInput")
    wc1_d = nc.dram_tensor("wc1", (128, 6, 2, 128), dt.float8e4, kind="ExternalInput")
    wc2_d = nc.dram_tensor("wc2", (128, 6, 2, 128), dt.float8e4, kind="ExternalInput")
    wh_d = nc.dram_tensor("wh", (128, 3, 2, 16), dt.float8e4, kind="ExternalInput")
    y_d = nc.dram_tensor("y", (1, COLS), dt.float32, kind="ExternalOutput")

    CPG = GROUP // CHUNK      # chunks per group

    with tile.TileContext(nc) as tc:
        with (
            tc.tile_pool(name="weights", bufs=1) as wpool,
            tc.tile_pool(name="xin", bufs=3) as xpool,
            tc.tile_pool(name="h1", bufs=2) as h1pool,
            tc.tile_pool(name="h2", bufs=2) as h2pool,
            tc.tile_pool(name="yout", bufs=2) as ypool,
            tc.tile_pool(name="ps1", bufs=4, space="PSUM") as ps1pool,
            tc.tile_pool(name="ps2", bufs=3, space="PSUM") as ps2pool,
            tc.tile_pool(name="psh", bufs=1, space="PSUM") as pshpool,
        ):
            # weights first on the sync queue; chunk-0 x split across the
            # scalar+gpsimd queues so conv1's first matmul starts ASAP
            wc1 = wpool.tile([128, 6, 2, 128], dt.float8e4, tag="wc1")
            wc2 = wpool.tile([128, 6, 2, 128], dt.float8e4, tag="wc2")
            # M=1 padded to 16 so DoubleRow ldweights half-stride is 16B
            wh = wpool.tile([128, 3, 2, 16], dt.float8e4, tag="wh")
            # split the critical startup loads (wc1, x0 tiles 0-1) into
            # partition halves across queues: DMA flight time here is
            # descriptor-rate bound (~36ns/partition-row), so halving the
            # rows per queue halves time-to-data
            nc.sync.dma_start(wc1[0:64], wc1_d[0:64])
            nc.scalar.dma_start(wc1[64:128], wc1_d[64:128])

            x0_tile = xpool.tile([128, 6, CHUNK], dt.float8e4,
                                 tag="x0", name="x0")
            nc.gpsimd.dma_start(x0_tile[0:64, 0, :], xt_d[0:64, 0:CHUNK])
            nc.sync.dma_start(x0_tile[64:128, 0, :], xt_d[64:128, 0:CHUNK])
            nc.scalar.dma_start(x0_tile[0:64, 1, :], xt_d[128:192, 0:CHUNK])
            nc.gpsimd.dma_start(x0_tile[64:128, 1, :], xt_d[192:256, 0:CHUNK])
            q0 = [nc.sync, nc.scalar, nc.gpsimd, nc.sync]
            for t in range(2, 6):
                q0[t - 2].dma_start(
                    x0_tile[:, t, :],
                    xt_d[128 * t:128 * t + 128, 0:CHUNK],
                )
            nc.scalar.dma_start(wc2[:], wc2_d[:])
            nc.gpsimd.dma_start(wh[:], wh_d[:])
            x_tiles = [None] * N_GROUPS
            x_tiles[0] = xpool.tile([128, 6, GROUP], dt.float8e4,
                                    tag="x", name="x_0")

            def drain(eng, dst, ps, bf):
                # GPSIMD can't read PSUM: Scalar/Vector only.
                if eng % 2 == 0:
                    nc.scalar.activation(dst, ps, relu, bias=bf)
                else:
                    if bf == 0.0:
                        nc.vector.tensor_scalar_max(dst, ps, 0.0)
                    else:
                        nc.vector.tensor_scalar(
                            dst, ps, bf, 0.0,
                            mybir.AluOpType.add, mybir.AluOpType.max)

            h1_t = [None] * N_CHUNKS
            h2_t = [None] * N_CHUNKS
            y_t = [None] * N_GROUPS

            def emit_conv1(c):
                g = c // CPG
                if c == 1:
                    # group-0 rest: emitted after chunk 0's matmuls so the
                    # first matmul's DMA-semaphore target excludes these
                    for t in range(6):
                        nc.sync.dma_start(
                            x_tiles[0][:, t, CHUNK:GROUP],
                            xt_d[128 * t:128 * t + 128, CHUNK:GROUP],
                        )
                # prefetch group g+1 one chunk into group g (xin bufs=3)
                gn = g + 1
                if c % CPG == 1 and gn < N_GROUPS:
                    x_tiles[gn] = xpool.tile([128, 6, GROUP], dt.float8e4,
                                             tag="x", name=f"x_{gn}")
                    for t in range(6):
                        nc.sync.dma_start(
                            x_tiles[gn][:, t, :],
                            xt_d[128 * t:128 * t + 128,
                                 gn * GROUP:(gn + 1) * GROUP],
                        )
                if c == 0:
                    cs = slice(0, CHUNK)
                    xg = x0_tile
                else:
                    cs = slice((c % CPG) * CHUNK, (c % CPG + 1) * CHUNK)
                    xg = x_tiles[g]
                h1_t[c] = h1pool.tile([128, 6, CHUNK], dt.float8e4,
                                      tag="h1", name=f"h1_{c}")
                return xg, cs

            def emit_conv1_mm(c, s, xg, cs):
                ob, k = CONV1_SPECS[s]
                ps = ps1pool.tile([128, CHUNK], dt.float32,
                                  tag="ps1", name=f"ps1_{c}_{s}")
                nc.tensor.matmul(
                    ps[:], wc1[:, s, :, :], xg[:, k:k + 2, cs],
                    start=True, stop=True, perf_mode=DR,
                )
                drain(s + c, h1_t[c][:, s, :], ps[:], b1f)

            def emit_conv2_prep(c):
                h2_t[c] = h2pool.tile([128, 6, CHUNK], dt.float8e4,
                                      tag="h2", name=f"h2_{c}")

            def emit_conv2_mm(c, t):
                a = CONV2_SLOT[t]
                ps = ps2pool.tile([128, CHUNK], dt.float32,
                                  tag="ps2", name=f"ps2_{c}_{t}")
                nc.tensor.matmul(
                    ps[:], wc2[:, t, :, :], h1_t[c][:, a:a + 2, :],
                    start=True, stop=True, perf_mode=DR,
                )
                drain(t + c + 1, h2_t[c][:, t, :], ps[:], b2f)

            def emit_head(c):
                g = c // CPG
                h2 = h2_t[c]
                if c % CPG == 0:
                    y_t[g] = ypool.tile([1, GROUP], dt.float32,
                                        tag="y", name=f"y_{g}")
                psh = pshpool.tile([1, CHUNK], dt.float32, tag="psh",
                                   name=f"psh_{c}")
                for k in range(3):
                    nc.tensor.matmul(
                        psh[:], wh[:, k, :, 0:1], h2[:, 2 * k:2 * k + 2, :],
                        start=(k == 0), stop=(k == 2), perf_mode=DR,
                    )
                ys = y_t[g][0:1, (c % CPG) * CHUNK:(c % CPG + 1) * CHUNK]
                if c % 2 == 0:
                    nc.scalar.activation(ys, psh[:], relu, bias=linbf)
                else:
                    if linbf == 0.0:
                        nc.vector.tensor_scalar_max(ys, psh[:], 0.0)
                    else:
                        nc.vector.tensor_scalar(
                            ys, psh[:], linbf, 0.0,
                            mybir.AluOpType.add, mybir.AluOpType.max)
                h2_t[c] = None
                if c % CPG == CPG - 1:
                    nc.sync.dma_start(
                        y_d[0:1, g * GROUP:(g + 1) * GROUP], y_t[g][:],
                    )

            # software pipeline: conv1(c) | conv2(c-1) | head(c-2), with
            # conv1/conv2 matmuls INTERLEAVED so a PSUM buffer's gating
            # drain is issued 6 matmul-slots (not 3) before its reuse
            for cc in range(N_CHUNKS + 2):
                xg = cs = None
                if cc < N_CHUNKS:
                    xg, cs = emit_conv1(cc)
                if 1 <= cc <= N_CHUNKS:
                    emit_conv2_prep(cc - 1)
                for s in range(6):
                    if cc < N_CHUNKS:
                        emit_conv1_mm(cc, s, xg, cs)
                    if 1 <= cc <= N_CHUNKS:
                        emit_conv2_mm(cc - 1, s)
                if cc >= 2:
                    emit_head(cc - 2)

    nc.compile()
    _PROGRAM_CACHE[key] = nc
    return nc


def _kernel_fp8(x, A, w1f, b1f, w2f, b2f, lw, linbf):
    global LAST_RESULT
    from concourse import bass_utils

    wc1_np, wc2_np, wh_np = _pack_fp8_weights(
        (w1f * A).astype(np.float32), (w2f * A).astype(np.float32), lw)

    nc = _build_fp8_program(b1f, b2f, linbf)

    xt = np.zeros((NP, B_TOTAL), e4m3)
    xt[:N] = np.ascontiguousarray(x.T).astype(e4m3)
    in_maps = []
    for c in range(N_CORES):
        in_maps.append({
            "xt": np.ascontiguousarray(xt[:, c * COLS:(c + 1) * COLS]),
            "wc1": wc1_np,
            "wc2": wc2_np,
            "wh": wh_np,
        })

    res = bass_utils.run_bass_kernel_spmd(
        nc, in_maps, list(range(N_CORES)), trace=TRACE
    )
    if TRACE:
        LAST_RESULT = res
    out = np.concatenate([res.results[c]["y"].reshape(-1) for c in range(N_CORES)])
    return out.reshape(B_TOTAL, 1).astype(np.float32)


# ---------------------------------------------------------------------------
# bf16 block-tridiagonal fallback (general inputs)
# ---------------------------------------------------------------------------

def _neighbors(m):
    return [k for k in (m - 1, m, m + 1) if 0 <= k < N_TILES]


_BOFF = {}
_W = 0
for _m in range(N_TILES):
    for _k in _neighbors(_m):
        _BOFF[(_m, _k)] = _W
        _W += P[_m]


DIAG_OFF = [sum(P[:m]) for m in range(N_TILES)]
DIAG_W = sum(P)
COR_W = 4 * 128 + 128 + P[-1]    # 4 packed pairs + lo(4) + full-K hi(5)
LO4_OFF = 4 * 128
HI5_OFF = 5 * 128


def _pack_blocks(Bmat):
    """Pack lhsT blocks of the block-tridiagonal operator.

    diag [128, 676]: block m = Bmat[tile m, tile m] at DIAG_OFF[m].
    cor  [128, 676]:
      pair i (i=0..3) at cols 128*i, shared column range:
        rows  0:32  -> lo(i):  first-32-rows window of tile i+1 -> out-tile i
        rows 64:128 -> hi(i+1): last-64-rows window of tile i  -> out-tile i+1
      (the two run concurrently in disjoint PE row groups)
      lo(4) at cols LO4_OFF (rows 0:32 of tile 5 -> out-tile 4)
      hi(5) at cols HI5_OFF: FULL-K block of tile 4 -> out-tile 5
        (K=64/base-64 into an M=36 psum hard-faults the HW - probed)
    """
    diag = np.zeros((128, DIAG_W), dtype=bf16)
    for m in range(N_TILES):
        blk = Bmat[OFF[m]:OFF[m] + P[m], OFF[m]:OFF[m] + P[m]]
        diag[: P[m], DIAG_OFF[m]:DIAG_OFF[m] + P[m]] = blk.astype(bf16)
    cor = np.zeros((128, COR_W), dtype=bf16)
    for i in range(4):
        c = 128 * i
        cor[0:32, c:c + 128] = Bmat[OFF[i + 1]:OFF[i + 1] + 32,
                                    OFF[i]:OFF[i] + 128].astype(bf16)
        cor[64:128, c:c + 128] = Bmat[OFF[i + 1] - 64:OFF[i + 1],
                                      OFF[i + 1]:OFF[i + 1] + 128].astype(bf16)
    cor[0:32, LO4_OFF:LO4_OFF + 128] = Bmat[OFF[5]:OFF[5] + 32,
                                            OFF[4]:OFF[4] + 128].astype(bf16)
    cor[0:128, HI5_OFF:HI5_OFF + P[5]] = Bmat[OFF[4]:OFF[4] + 128,
                                              OFF[5]:OFF[5] + P[5]].astype(bf16)
    return diag, cor


def _build_program(b1f, b2f, linbf):
    key = (b1f, b2f, linbf)
    if key in _PROGRAM_CACHE:
        return _PROGRAM_CACHE[key]

    import concourse.mybir as mybir
    import concourse.tile as tile
    from concourse import bacc

    nc = bacc.Bacc(None, target_bir_lowering=False)
    dt = mybir.dt

    xt_d = nc.dram_tensor("xt", (N, COLS), dt.bfloat16, kind="ExternalInput")
    wd1_d = nc.dram_tensor("wd1", (128, DIAG_W), dt.bfloat16, kind="ExternalInput")
    wd2_d = nc.dram_tensor("wd2", (128, DIAG_W), dt.bfloat16, kind="ExternalInput")
    wr1_d = nc.dram_tensor("wr1", (128, COR_W), dt.bfloat16, kind="ExternalInput")
    wr2_d = nc.dram_tensor("wr2", (128, COR_W), dt.bfloat16, kind="ExternalInput")
    wlin_d = nc.dram_tensor("wlin", (128, N_TILES), dt.bfloat16, kind="ExternalInput")
    y_d = nc.dram_tensor("y", (1, COLS), dt.float32, kind="ExternalOutput")

    with tile.TileContext(nc) as tc:
        with (
            tc.tile_pool(name="weights", bufs=1) as wpool,
            tc.tile_pool(name="xin", bufs=3) as xpool,
            tc.tile_pool(name="acts", bufs=2) as hpool,
            tc.tile_pool(name="yout", bufs=1) as ypool,
            tc.tile_pool(name="ps1", bufs=3, space="PSUM") as ps1pool,
            tc.tile_pool(name="ps2", bufs=3, space="PSUM") as ps2pool,
            tc.tile_pool(name="psl", bufs=2, space="PSUM") as pslpool,
        ):
            # x chunk 0 first so compute starts ASAP, then weights, then rest
            xt_tiles = [[None] * N_GROUPS for _ in range(N_TILES)]
            for t in range(N_TILES):
                xt_tiles[t][0] = xpool.tile([P[t], GROUP], dt.bfloat16,
                                            tag=f"x{t}", name=f"x{t}_0")
                nc.sync.dma_start(
                    xt_tiles[t][0][:, 0:CHUNK],
                    xt_d[OFF[t]:OFF[t] + P[t], 0:CHUNK],
                )

            wd1 = wpool.tile([128, DIAG_W], dt.bfloat16, tag="wd1")
            wd2 = wpool.tile([128, DIAG_W], dt.bfloat16, tag="wd2")
            wr1 = wpool.tile([128, COR_W], dt.bfloat16, tag="wr1")
            wr2 = wpool.tile([128, COR_W], dt.bfloat16, tag="wr2")
            wlin = wpool.tile([128, N_TILES], dt.bfloat16, tag="wlin")
            nc.sync.dma_start(wd1[:], wd1_d[:])
            nc.sync.dma_start(wd2[:], wd2_d[:])
            nc.sync.dma_start(wr1[:], wr1_d[:])
            nc.sync.dma_start(wr2[:], wr2_d[:])
            nc.sync.dma_start(wlin[:], wlin_d[:])

            for t in range(N_TILES):
                nc.sync.dma_start(
                    xt_tiles[t][0][:, CHUNK:GROUP],
                    xt_d[OFF[t]:OFF[t] + P[t], CHUNK:GROUP],
                )

            y_sb = ypool.tile([1, COLS], dt.float32, tag="y")
            relu = mybir.ActivationFunctionType.Relu

            for c in range(N_CHUNKS):
                g = c // (GROUP // CHUNK)
                if c % (GROUP // CHUNK) == 0 and g > 0:
                    for t in range(N_TILES):
                        xt_tiles[t][g] = xpool.tile(
                            [P[t], GROUP], dt.bfloat16, tag=f"x{t}",
                            name=f"x{t}_{g}",
                        )
                        nc.sync.dma_start(
                            xt_tiles[t][g][:],
                            xt_d[OFF[t]:OFF[t] + P[t],
                                 g * GROUP:(g + 1) * GROUP],
                        )
                cs = slice((c % (GROUP // CHUNK)) * CHUNK,
                           (c % (GROUP // CHUNK) + 1) * CHUNK)

                def emit_conv(wd, wr, rhs_of, pspool, pstag, drain):
                    """6 diag MMs + packed corner pairs (disjoint 32/64-row
                    PE groups run concurrently) + full-K m=5 corner."""
                    ps = [None] * N_TILES
                    for m in range(N_TILES):
                        ps[m] = pspool.tile([P[m], CHUNK], dt.float32,
                                            tag=pstag, name=f"{pstag}_{m}")
                        nc.tensor.matmul(
                            ps[m][:],
                            wd[: P[m], DIAG_OFF[m]:DIAG_OFF[m] + P[m]],
                            rhs_of(m),
                            start=True, stop=False,
                        )
                        if 1 <= m <= 4:
                            i = m - 1
                            nc.tensor.matmul(          # lo(i) closes psum i
                                ps[i][:],
                                wr[0:32, 128 * i:128 * i + 128],
                                rhs_of(m)[0:32, :],
                                start=False, stop=True,
                            )
                            nc.tensor.matmul(          # hi(m), rows 64:128
                                ps[m][:],
                                wr[64:128, 128 * i:128 * i + 128],
                                rhs_of(i)[64:128, :],
                                start=False, stop=False,
                            )
                            drain(i, ps[i])
                        elif m == 5:
                            nc.tensor.matmul(          # lo(4) closes psum 4
                                ps[4][:],
                                wr[0:32, LO4_OFF:LO4_OFF + 128],
                                rhs_of(5)[0:32, :],
                                start=False, stop=True,
                            )
                            nc.tensor.matmul(          # hi(5) full-K
                                ps[5][:],
                                wr[0:128, HI5_OFF:HI5_OFF + P[5]],
                                rhs_of(4),
                                start=False, stop=True,
                            )
                            drain(4, ps[4])
                            drain(5, ps[5])

                # ---- conv1: h1 = relu(B1 @ xT + b1) ----
                h1 = [None] * N_TILES

                def drain1(m, ps):
                    h = hpool.tile([P[m], CHUNK], dt.bfloat16,
                                   tag=f"h1_{m}", name=f"h1_{m}")
                    nc.scalar.activation(h[:], ps[:], relu, bias=b1f)
                    h1[m] = h

                emit_conv(wd1, wr1, lambda k: xt_tiles[k][g][:, cs],
                          ps1pool, "ps1", drain1)

                # ---- conv2: h2 = relu(B2 @ h1 + b2) ----
                h2 = [None] * N_TILES

                def drain2(m, ps):
                    h = hpool.tile([P[m], CHUNK], dt.bfloat16,
                                   tag=f"h2_{m}", name=f"h2_{m}")
                    if b2f == 0.0:
                        nc.vector.tensor_scalar_max(h[:], ps[:], 0.0)
                    else:
                        nc.vector.tensor_scalar(
                            h[:], ps[:], b2f, 0.0,
                            mybir.AluOpType.add, mybir.AluOpType.max,
                        )
                    h2[m] = h

                emit_conv(wd2, wr2, lambda k: h1[k][:],
                          ps2pool, "ps2", drain2)

                # ---- linear head: y = relu(linw.T @ h2 + lin_b) ----
                psl = pslpool.tile([1, CHUNK], dt.float32, tag="psl",
                                   name="psl")
                for k in range(N_TILES):
                    nc.tensor.matmul(
                        psl[:],
                        wlin[: P[k], k:k + 1],
                        h2[k][:],
                        start=(k == 0),
                        stop=(k == N_TILES - 1),
                    )
                nc.scalar.activation(
                    y_sb[0:1, c * CHUNK:(c + 1) * CHUNK], psl[:], relu,
                    bias=linbf,
                )

            nc.sync.dma_start(y_d[:], y_sb[:])

    nc.compile()
    _PROGRAM_CACHE[key] = nc
    return nc


def _kernel_bf16(x, A, w1f, b1f, w2f, b2f, lw, linbf):
    global LAST_RESULT
    from concourse import bass_utils

    wd1_np, wr1_np = _pack_blocks((w1f * A).astype(np.float32))
    wd2_np, wr2_np = _pack_blocks((w2f * A).astype(np.float32))
    wlin_np = np.zeros((128, N_TILES), dtype=bf16)
    for t in range(N_TILES):
        wlin_np[: P[t], t] = lw[OFF[t]:OFF[t] + P[t]].astype(bf16)

    nc = _build_program(b1f, b2f, linbf)

    # host-side: transpose, cast, shard along batch
    xt = np.ascontiguousarray(x.T).astype(bf16)        # [676, 65536]
    in_maps = []
    for c in range(N_CORES):
        in_maps.append({
            "xt": np.ascontiguousarray(xt[:, c * COLS:(c + 1) * COLS]),
            "wd1": wd1_np,
            "wd2": wd2_np,
            "wr1": wr1_np,
            "wr2": wr2_np,
            "wlin": wlin_np,
        })

    res = bass_utils.run_bass_kernel_spmd(
        nc, in_maps, list(range(N_CORES)), trace=TRACE
    )
    if TRACE:
        LAST_RESULT = res
    out = np.concatenate([res.results[c]["y"].reshape(-1) for c in range(N_CORES)])
    return out.reshape(B_TOTAL, 1).astype(np.float32)


def kernel(x, w1, b1, w2, b2, lin_w, lin_b, edge_src, edge_dst):
    import os

    x = np.asarray(x)
    # Build the dense normalized aggregation operator from the edge lists.
    deg = np.zeros(N, np.float64)
    np.add.at(deg, np.asarray(edge_dst), 1.0)
    dinv = 1.0 / np.sqrt(deg)
    normv = dinv[np.asarray(edge_src)] * dinv[np.asarray(edge_dst)]
    A = np.zeros((N, N), np.float64)
    np.add.at(A, (np.asarray(edge_dst), np.asarray(edge_src)), normv)

    w1f = float(np.asarray(w1).reshape(-1)[0])
    w2f = float(np.asarray(w2).reshape(-1)[0])
    b1f = float(np.asarray(b1).reshape(-1)[0])
    b2f = float(np.asarray(b2).reshape(-1)[0])
    linbf = float(np.asarray(lin_b).reshape(-1)[0])
    lw = np.asarray(lin_w).reshape(-1)

    # fp8 is bit-exact when w2 <= 0 and b2 <= 0 (h2 == 0 identically);
    # otherwise fall back to the bf16 kernel (~1e-3).
    use_fp8 = (w2f <= 0.0 and b2f <= 0.0) or os.environ.get("KERNEL_FORCE_FP8")
    if use_fp8:
        return _kernel_fp8(x, A, w1f, b1f, w2f, b2f, lw, linbf)
    return _kernel_bf16(x, A, w1f, b1f, w2f, b2f, lw, linbf)


# revision 29
# speedup vs baseline: 1.0020x; 1.0020x over previous
"""Trainium2 Bass kernel for the 2-layer grid-GCN + linear head.

Math: the GCN aggregation over the fixed graph is a linear operator on
the node axis: out = A @ h per batch column, where
A[j, i] = sum_{edges (i->j)} dinv[i]*dinv[j].  For the 26x26 grid with
row-major node order A is banded (|i-j| <= 26).  The whole network is

    h1 = relu(B1 @ xT + b1)      B1 = w1 * A
    h2 = relu(B2 @ h1 + b2)      B2 = w2 * A
    y  = relu(linw.T @ h2 + lin_b)

Batch is sharded across the 8 NeuronCores (pure data parallel).

Fast path (fp8 DoubleRow): when w2 <= 0 and b2 <= 0, the network output
is EXACTLY relu(lin_b) for any x/w1/b1: h1 >= 0 (relu), A >= 0
entrywise, so w2*A@h1 + b2 <= 0 and h2 == 0 identically — in any
rounding mode that preserves sign (fp8 does).  So fp8 precision is
bit-exact on such instances while the tensor engine runs DoubleRow
matmuls (2 rhs rows/cycle, K=256 per instruction):

  conv1: 6 DoubleRow matmuls; output tiles = aligned [0,128) plus
         shifted [64+128j, 192+128j) so every tile's +-26 dependency
         band fits an aligned K=256 window of x tiles.
  conv2: 6 DoubleRow matmuls back to ALIGNED output tiles, reading
         contiguous pairs of the shifted h1 slots.
  head:  3 DoubleRow matmuls (K=256 each) accumulating one PSUM.

PSUM->SBUF relu+cast drains alternate Scalar/Vector (GPSIMD cannot
read PSUM); emission is software-pipelined conv1(c)|conv2(c-1)|head(c-2)
with conv1/conv2 matmuls interleaved for PSUM-reuse slack.

General fallback (w2 > 0 or b2 > 0): the bf16 block-tridiagonal kernel
(rel err ~1e-3), below.
"""

import sys

if "/opt/trn_rl_repo" not in sys.path:
    sys.path.insert(0, "/opt/trn_rl_repo")

import numpy as np
import ml_dtypes

N_CORES = 8
N = 676           # nodes (26x26 grid)
NP = 768          # padded to 6 x 128
B_TOTAL = 65536
COLS = B_TOTAL // N_CORES      # batch columns per core
CHUNK = 512                    # matmul free dim / PSUM bank
GROUP = 2048                   # DMA column-group
N_CHUNKS = COLS // CHUNK
N_GROUPS = COLS // GROUP
N_TILES = (N + 127) // 128     # 6 node tiles
P = [min(128, N - 128 * t) for t in range(N_TILES)]   # [128]*5 + [36]
OFF = [128 * t for t in range(N_TILES)]

bf16 = ml_dtypes.bfloat16
e4m3 = ml_dtypes.float8_e4m3

TRACE = False            # test.py flips this to profile
LAST_RESULT = None       # BassKernelResults stash when TRACE

# ---------------------------------------------------------------------------
# fp8 DoubleRow fast path
# ---------------------------------------------------------------------------

# h1 slot s covers nodes [SLOT_BASE[s], SLOT_BASE[s]+128)
SLOT_BASE = [0, 64, 192, 320, 448, 576]
# conv1 matmul s: (out_base, x-tile index k -> halves (k, k+1))
CONV1_SPECS = [(0, 0)] + [(64 + 128 * j, j) for j in range(5)]
# conv2 out tile t reads h1 slot pair (a, a+1); a per tile:
CONV2_SLOT = [0, 1, 2, 3, 4, 4]


def _pack_fp8_weights(B1, B2, lw):
    """lhsT blocks [128, 2, 128]: w[p, h, m] = B[ob+m, ib_h+p]."""

    def make_w(B, ob, ibs, t0rule):
        w = np.zeros((128, 2, 128), np.float32)
        for h, ib in enumerate(ibs):
            ipmax = min(128, N - ib)
            ommax = min(128, N - ob)
            if ipmax > 0 and ommax > 0:
                w[:ipmax, h, :ommax] = B[ob:ob + ommax, ib:ib + ipmax].T
        if t0rule:  # conv2 t=0: in-nodes [64,128) owned by half0
            w[:64, 1, :] = 0.0
        return w

    wc1 = np.zeros((128, 6, 2, 128), e4m3)
    for s, (ob, k) in enumerate(CONV1_SPECS):
        wc1[:, s] = make_w(B1, ob, (128 * k, 128 * k + 128), False).astype(e4m3)
    wc2 = np.zeros((128, 6, 2, 128), e4m3)
    for t in range(6):
        a = CONV2_SLOT[t]
        wc2[:, t] = make_w(
            B2, 128 * t, (SLOT_BASE[a], SLOT_BASE[a + 1]), t == 0
        ).astype(e4m3)
    wh = np.zeros((128, 3, 2, 16), e4m3)
    lwp = np.zeros(NP, np.float32)
    lwp[:N] = lw
    for k in range(3):
        for h in range(2):
            base = 256 * k + 128 * h
            wh[:, k, h, 0] = lwp[base:base + 128].astype(e4m3)
    return wc1, wc2, wh


_PROGRAM_CACHE = {}


def _build_fp8_program(b1f, b2f, linbf):
    key = ("fp8", b1f, b2f, linbf)
    if key in _PROGRAM_CACHE:
        return _PROGRAM_CACHE[key]

    import concourse.mybir as mybir
    import concourse.tile as tile
    from concourse import bacc

    nc = bacc.Bacc(None, target_bir_lowering=False)
    dt = mybir.dt
    DR = mybir.MatmulPerfMode.DoubleRow
    relu = mybir.ActivationFunctionType.Relu

    xt_d = nc.dram_tensor("xt", (NP, COLS), dt.float8e4, kind="ExternalInput")
    wc1_d = nc.dram_tensor("wc1", (128, 6, 2, 128), dt.float8e4, kind="ExternalInput")
    wc2_d = nc.dram_tensor("wc2", (128, 6, 2, 128), dt.float8e4, kind="ExternalInput")
    wh_d = nc.dram_tensor("wh", (128, 3, 2, 16), dt.float8e4, kind="ExternalInput")
    y_d = nc.dram_tensor("y", (1, COLS), dt.float32, kind="ExternalOutput")

    CPG = GROUP // CHUNK      # chunks per group

    with tile.TileContext(nc) as tc:
        with (
            tc.tile_pool(name="weights", bufs=1) as wpool,
            tc.tile_pool(name="xin", bufs=3) as xpool,
            tc.tile_pool(name="h1", bufs=2) as h1pool,
            tc.tile_pool(name="h2", bufs=2) as h2pool,
            tc.tile_pool(name="yout", bufs=2) as ypool,
            tc.tile_pool(name="ps1", bufs=3, space="PSUM") as ps1pool,
            tc.tile_pool(name="ps2", bufs=3, space="PSUM") as ps2pool,
            tc.tile_pool(name="psh", bufs=2, space="PSUM") as pshpool,
        ):
            # weights first on the sync queue; chunk-0 x split across the
            # scalar+gpsimd queues so conv1's first matmul starts ASAP
            wc1 = wpool.tile([128, 6, 2, 128], dt.float8e4, tag="wc1")
            wc2 = wpool.tile([128, 6, 2, 128], dt.float8e4, tag="wc2")
            # M=1 padded to 16 so DoubleRow ldweights half-stride is 16B
            wh = wpool.tile([128, 3, 2, 16], dt.float8e4, tag="wh")
            # split the critical startup loads (wc1, x0 tiles 0-1) into
            # partition halves across queues: DMA flight time here is
            # descriptor-rate bound (~36ns/partition-row), so halving the
            # rows per queue halves time-to-data
            nc.sync.dma_start(wc1[0:64], wc1_d[0:64])
            nc.scalar.dma_start(wc1[64:128], wc1_d[64:128])

            x0_tile = xpool.tile([128, 6, CHUNK], dt.float8e4,
                                 tag="x0", name="x0")
            nc.gpsimd.dma_start(x0_tile[0:64, 0, :], xt_d[0:64, 0:CHUNK])
            nc.sync.dma_start(x0_tile[64:128, 0, :], xt_d[64:128, 0:CHUNK])
            nc.scalar.dma_start(x0_tile[0:64, 1, :], xt_d[128:192, 0:CHUNK])
            nc.gpsimd.dma_start(x0_tile[64:128, 1, :], xt_d[192:256, 0:CHUNK])
            q0 = [nc.sync, nc.scalar, nc.gpsimd, nc.sync]
            for t in range(2, 6):
                q0[t - 2].dma_start(
                    x0_tile[:, t, :],
                    xt_d[128 * t:128 * t + 128, 0:CHUNK],
                )
            nc.scalar.dma_start(wc2[:], wc2_d[:])
            nc.gpsimd.dma_start(wh[:], wh_d[:])
            x_tiles = [None] * N_GROUPS
            x_tiles[0] = xpool.tile([128, 6, GROUP], dt.float8e4,
                                    tag="x", name="x_0")

            def drain(eng, dst, ps, bf):
                # GPSIMD can't read PSUM: Scalar/Vector only.
                if eng % 2 == 0:
                    nc.scalar.activation(dst, ps, relu, bias=bf)
                else:
                    if bf == 0.0:
                        nc.vector.tensor_scalar_max(dst, ps, 0.0)
                    else:
                        nc.vector.tensor_scalar(
                            dst, ps, bf, 0.0,
                            mybir.AluOpType.add, mybir.AluOpType.max)

            h1_t = [None] * N_CHUNKS
            h2_t = [None] * N_CHUNKS
            y_t = [None] * N_GROUPS

            def emit_conv1(c):
                g = c // CPG
                if c == 1:
                    # group-0 rest: emitted after chunk 0's matmuls so the
                    # first matmul's DMA-semaphore target excludes these
                    for t in range(6):
                        nc.sync.dma_start(
                            x_tiles[0][:, t, CHUNK:GROUP],
                            xt_d[128 * t:128 * t + 128, CHUNK:GROUP],
                        )
                # prefetch group g+1 one chunk into group g (xin bufs=3)
                gn = g + 1
                if c % CPG == 1 and gn < N_GROUPS:
                    x_tiles[gn] = xpool.tile([128, 6, GROUP], dt.float8e4,
                                             tag="x", name=f"x_{gn}")
                    for t in range(6):
                        nc.sync.dma_start(
                            x_tiles[gn][:, t, :],
                            xt_d[128 * t:128 * t + 128,
                                 gn * GROUP:(gn + 1) * GROUP],
                        )
                if c == 0:
                    cs = slice(0, CHUNK)
                    xg = x0_tile
                else:
                    cs = slice((c % CPG) * CHUNK, (c % CPG + 1) * CHUNK)
                    xg = x_tiles[g]
                h1_t[c] = h1pool.tile([128, 6, CHUNK], dt.float8e4,
                                      tag="h1", name=f"h1_{c}")
                return xg, cs

            def emit_conv1_mm(c, s, xg, cs):
                ob, k = CONV1_SPECS[s]
                ps = ps1pool.tile([128, CHUNK], dt.float32,
                                  tag="ps1", name=f"ps1_{c}_{s}")
                nc.tensor.matmul(
                    ps[:], wc1[:, s, :, :], xg[:, k:k + 2, cs],
                    start=True, stop=True, perf_mode=DR,
                )
                drain(s + c, h1_t[c][:, s, :], ps[:], b1f)

            def emit_conv2_prep(c):
                h2_t[c] = h2pool.tile([128, 6, CHUNK], dt.float8e4,
                                      tag="h2", name=f"h2_{c}")

            def emit_conv2_mm(c, t):
                a = CONV2_SLOT[t]
                ps = ps2pool.tile([128, CHUNK], dt.float32,
                                  tag="ps2", name=f"ps2_{c}_{t}")
                nc.tensor.matmul(
                    ps[:], wc2[:, t, :, :], h1_t[c][:, a:a + 2, :],
                    start=True, stop=True, perf_mode=DR,
                )
                drain(t + c + 1, h2_t[c][:, t, :], ps[:], b2f)

            def emit_head(c):
                g = c // CPG
                h2 = h2_t[c]
                if c % CPG == 0:
                    y_t[g] = ypool.tile([1, GROUP], dt.float32,
                                        tag="y", name=f"y_{g}")
                psh = pshpool.tile([1, CHUNK], dt.float32, tag="psh",
                                   name=f"psh_{c}")
                for k in range(3):
                    nc.tensor.matmul(
                        psh[:], wh[:, k, :, 0:1], h2[:, 2 * k:2 * k + 2, :],
                        start=(k == 0), stop=(k == 2), perf_mode=DR,
                    )
                ys = y_t[g][0:1, (c % CPG) * CHUNK:(c % CPG + 1) * CHUNK]
                if c % 2 == 0:
                    nc.scalar.activation(ys, psh[:], relu, bias=linbf)
                else:
                    if linbf == 0.0:
                        nc.vector.tensor_scalar_max(ys, psh[:], 0.0)
                    else:
                        nc.vector.tensor_scalar(
                            ys, psh[:], linbf, 0.0,
                            mybir.AluOpType.add, mybir.AluOpType.max)
                h2_t[c] = None
                if c % CPG == CPG - 1:
                    nc.sync.dma_start(
                        y_d[0:1, g * GROUP:(g + 1) * GROUP], y_t[g][:],
                    )

            # software pipeline: conv1(c) | conv2(c-1) | head(c-2), with
            # conv1/conv2 matmuls INTERLEAVED so a PSUM buffer's gating
            # drain is issued 6 matmul-slots (not 3) before its reuse
            for cc in range(N_CHUNKS + 2):
                xg = cs = None
                if cc < N_CHUNKS:
                    xg, cs = emit_conv1(cc)
                if 1 <= cc <= N_CHUNKS:
                    emit_conv2_prep(cc - 1)
                for s in range(6):
                    if cc < N_CHUNKS:
                        emit_conv1_mm(cc, s, xg, cs)
                    if 1 <= cc <= N_CHUNKS:
                        emit_conv2_mm(cc - 1, s)
                if cc >= 2:
                    emit_head(cc - 2)

    nc.compile()
    _PROGRAM_CACHE[key] = nc
    return nc


def _kernel_fp8(x, A, w1f, b1f, w2f, b2f, lw, linbf):
    global LAST_RESULT
    from concourse import bass_utils

    wc1_np, wc2_np, wh_np = _pack_fp8_weights(
        (w1f * A).astype(np.float32), (w2f * A).astype(np.float32), lw)

    nc = _build_fp8_program(b1f, b2f, linbf)

    xt = np.zeros((NP, B_TOTAL), e4m3)
    xt[:N] = np.ascontiguousarray(x.T).astype(e4m3)
    in_maps = []
    for c in range(N_CORES):
        in_maps.append({
            "xt": np.ascontiguousarray(xt[:, c * COLS:(c + 1) * COLS]),
            "wc1": wc1_np,
            "wc2": wc2_np,
            "wh": wh_np,
        })

    res = bass_utils.run_bass_kernel_spmd(
        nc, in_maps, list(range(N_CORES)), trace=TRACE
    )
    if TRACE:
        LAST_RESULT = res
    out = np.concatenate([res.results[c]["y"].reshape(-1) for c in range(N_CORES)])
    return out.reshape(B_TOTAL, 1).astype(np.float32)


# ---------------------------------------------------------------------------
# bf16 block-tridiagonal fallback (general inputs)
# ---------------------------------------------------------------------------

def _neighbors(m):
    return [k for k in (m - 1, m, m + 1) if 0 <= k < N_TILES]


_BOFF = {}
_W = 0
for _m in range(N_TILES):
    for _k in _neighbors(_m):
        _BOFF[(_m, _k)] = _W
        _W += P[_m]


DIAG_OFF = [sum(P[:m]) for m in range(N_TILES)]
DIAG_W = sum(P)
COR_W = 4 * 128 + 128 + P[-1]    # 4 packed pairs + lo(4) + full-K hi(5)
LO4_OFF = 4 * 128
HI5_OFF = 5 * 128


def _pack_blocks(Bmat):
    """Pack lhsT blocks of the block-tridiagonal operator.

    diag [128, 676]: block m = Bmat[tile m, tile m] at DIAG_OFF[m].
    cor  [128, 676]:
      pair i (i=0..3) at cols 128*i, shared column range:
        rows  0:32  -> lo(i):  first-32-rows window of tile i+1 -> out-tile i
        rows 64:128 -> hi(i+1): last-64-rows window of tile i  -> out-tile i+1
      (the two run concurrently in disjoint PE row groups)
      lo(4) at cols LO4_OFF (rows 0:32 of tile 5 -> out-tile 4)
      hi(5) at cols HI5_OFF: FULL-K block of tile 4 -> out-tile 5
        (K=64/base-64 into an M=36 psum hard-faults the HW - probed)
    """
    diag = np.zeros((128, DIAG_W), dtype=bf16)
    for m in range(N_TILES):
        blk = Bmat[OFF[m]:OFF[m] + P[m], OFF[m]:OFF[m] + P[m]]
        diag[: P[m], DIAG_OFF[m]:DIAG_OFF[m] + P[m]] = blk.astype(bf16)
    cor = np.zeros((128, COR_W), dtype=bf16)
    for i in range(4):
        c = 128 * i
        cor[0:32, c:c + 128] = Bmat[OFF[i + 1]:OFF[i + 1] + 32,
                                    OFF[i]:OFF[i] + 128].astype(bf16)
        cor[64:128, c:c + 128] = Bmat[OFF[i + 1] - 64:OFF[i + 1],
                                      OFF[i + 1]:OFF[i + 1] + 128].astype(bf16)
    cor[0:32, LO4_OFF:LO4_OFF + 128] = Bmat[OFF[5]:OFF[5] + 32,
                                            OFF[4]:OFF[4] + 128].astype(bf16)
    cor[0:128, HI5_OFF:HI5_OFF + P[5]] = Bmat[OFF[4]:OFF[4] + 128,
                                              OFF[5]:OFF[5] + P[5]].astype(bf16)
    return diag, cor


def _build_program(b1f, b2f, linbf):
    key = (b1f, b2f, linbf)
    if key in _PROGRAM_CACHE:
        return _PROGRAM_CACHE[key]

    import concourse.mybir as mybir
    import concourse.tile as tile
    from concourse import bacc

    nc = bacc.Bacc(None, target_bir_lowering=False)
    dt = mybir.dt

    xt_d = nc.dram_tensor("xt", (N, COLS), dt.bfloat16, kind="ExternalInput")
    wd1_d = nc.dram_tensor("wd1", (128, DIAG_W), dt.bfloat16, kind="ExternalInput")
    wd2_d = nc.dram_tensor("wd2", (128, DIAG_W), dt.bfloat16, kind="ExternalInput")
    wr1_d = nc.dram_tensor("wr1", (128, COR_W), dt.bfloat16, kind="ExternalInput")
    wr2_d = nc.dram_tensor("wr2", (128, COR_W), dt.bfloat16, kind="ExternalInput")
    wlin_d = nc.dram_tensor("wlin", (128, N_TILES), dt.bfloat16, kind="ExternalInput")
    y_d = nc.dram_tensor("y", (1, COLS), dt.float32, kind="ExternalOutput")

    with tile.TileContext(nc) as tc:
        with (
            tc.tile_pool(name="weights", bufs=1) as wpool,
            tc.tile_pool(name="xin", bufs=3) as xpool,
            tc.tile_pool(name="acts", bufs=2) as hpool,
            tc.tile_pool(name="yout", bufs=1) as ypool,
            tc.tile_pool(name="ps1", bufs=3, space="PSUM") as ps1pool,
            tc.tile_pool(name="ps2", bufs=3, space="PSUM") as ps2pool,
            tc.tile_pool(name="psl", bufs=2, space="PSUM") as pslpool,
        ):
            # x chunk 0 first so compute starts ASAP, then weights, then rest
            xt_tiles = [[None] * N_GROUPS for _ in range(N_TILES)]
            for t in range(N_TILES):
                xt_tiles[t][0] = xpool.tile([P[t], GROUP], dt.bfloat16,
                                            tag=f"x{t}", name=f"x{t}_0")
                nc.sync.dma_start(
                    xt_tiles[t][0][:, 0:CHUNK],
                    xt_d[OFF[t]:OFF[t] + P[t], 0:CHUNK],
                )

            wd1 = wpool.tile([128, DIAG_W], dt.bfloat16, tag="wd1")
            wd2 = wpool.tile([128, DIAG_W], dt.bfloat16, tag="wd2")
            wr1 = wpool.tile([128, COR_W], dt.bfloat16, tag="wr1")
            wr2 = wpool.tile([128, COR_W], dt.bfloat16, tag="wr2")
            wlin = wpool.tile([128, N_TILES], dt.bfloat16, tag="wlin")
            nc.sync.dma_start(wd1[:], wd1_d[:])
            nc.sync.dma_start(wd2[:], wd2_d[:])
            nc.sync.dma_start(wr1[:], wr1_d[:])
            nc.sync.dma_start(wr2[:], wr2_d[:])
            nc.sync.dma_start(wlin[:], wlin_d[:])

            for t in range(N_TILES):
                nc.sync.dma_start(
                    xt_tiles[t][0][:, CHUNK:GROUP],
                    xt_d[OFF[t]:OFF[t] + P[t], CHUNK:GROUP],
                )

            y_sb = ypool.tile([1, COLS], dt.float32, tag="y")
            relu = mybir.ActivationFunctionType.Relu

            for c in range(N_CHUNKS):
                g = c // (GROUP // CHUNK)
                if c % (GROUP // CHUNK) == 0 and g > 0:
                    for t in range(N_TILES):
                        xt_tiles[t][g] = xpool.tile(
                            [P[t], GROUP], dt.bfloat16, tag=f"x{t}",
                            name=f"x{t}_{g}",
                        )
                        nc.sync.dma_start(
                            xt_tiles[t][g][:],
                            xt_d[OFF[t]:OFF[t] + P[t],
                                 g * GROUP:(g + 1) * GROUP],
                        )
                cs = slice((c % (GROUP // CHUNK)) * CHUNK,
                           (c % (GROUP // CHUNK) + 1) * CHUNK)

                def emit_conv(wd, wr, rhs_of, pspool, pstag, drain):
                    """6 diag MMs + packed corner pairs (disjoint 32/64-row
                    PE groups run concurrently) + full-K m=5 corner."""
                    ps = [None] * N_TILES
                    for m in range(N_TILES):
                        ps[m] = pspool.tile([P[m], CHUNK], dt.float32,
                                            tag=pstag, name=f"{pstag}_{m}")
                        nc.tensor.matmul(
                            ps[m][:],
                            wd[: P[m], DIAG_OFF[m]:DIAG_OFF[m] + P[m]],
                            rhs_of(m),
                            start=True, stop=False,
                        )
                        if 1 <= m <= 4:
                            i = m - 1
                            nc.tensor.matmul(          # lo(i) closes psum i
                                ps[i][:],
                                wr[0:32, 128 * i:128 * i + 128],
                                rhs_of(m)[0:32, :],
                                start=False, stop=True,
                            )
                            nc.tensor.matmul(          # hi(m), rows 64:128
                                ps[m][:],
                                wr[64:128, 128 * i:128 * i + 128],
                                rhs_of(i)[64:128, :],
                                start=False, stop=False,
                            )
                            drain(i, ps[i])
                        elif m == 5:
                            nc.tensor.matmul(          # lo(4) closes psum 4
                                ps[4][:],
                                wr[0:32, LO4_OFF:LO4_OFF + 128],
                                rhs_of(5)[0:32, :],
                                start=False, stop=True,
                            )
                            nc.tensor.matmul(          # hi(5) full-K
                                ps[5][:],
                                wr[0:128, HI5_OFF:HI5_OFF + P[5]],
                                rhs_of(4),
                                start=False, stop=True,
                            )
                            drain(4, ps[4])
                            drain(5, ps[5])

                # ---- conv1: h1 = relu(B1 @ xT + b1) ----
                h1 = [None] * N_TILES

                def drain1(m, ps):
                    h = hpool.tile([P[m], CHUNK], dt.bfloat16,
                                   tag=f"h1_{m}", name=f"h1_{m}")
                    nc.scalar.activation(h[:], ps[:], relu, bias=b1f)
                    h1[m] = h

                emit_conv(wd1, wr1, lambda k: xt_tiles[k][g][:, cs],
                          ps1pool, "ps1", drain1)

                # ---- conv2: h2 = relu(B2 @ h1 + b2) ----
                h2 = [None] * N_TILES

                def drain2(m, ps):
                    h = hpool.tile([P[m], CHUNK], dt.bfloat16,
                                   tag=f"h2_{m}", name=f"h2_{m}")
                    if b2f == 0.0:
                        nc.vector.tensor_scalar_max(h[:], ps[:], 0.0)
                    else:
                        nc.vector.tensor_scalar(
                            h[:], ps[:], b2f, 0.0,
                            mybir.AluOpType.add, mybir.AluOpType.max,
                        )
                    h2[m] = h

                emit_conv(wd2, wr2, lambda k: h1[k][:],
                          ps2pool, "ps2", drain2)

                # ---- linear head: y = relu(linw.T @ h2 + lin_b) ----
                psl = pslpool.tile([1, CHUNK], dt.float32, tag="psl",
                                   name="psl")
                for k in range(N_TILES):
                    nc.tensor.matmul(
                        psl[:],
                        wlin[: P[k], k:k + 1],
                        h2[k][:],
                        start=(k == 0),
                        stop=(k == N_TILES - 1),
                    )
                nc.scalar.activation(
                    y_sb[0:1, c * CHUNK:(c + 1) * CHUNK], psl[:], relu,
                    bias=linbf,
                )

            nc.sync.dma_start(y_d[:], y_sb[:])

    nc.compile()
    _PROGRAM_CACHE[key] = nc
    return nc


def _kernel_bf16(x, A, w1f, b1f, w2f, b2f, lw, linbf):
    global LAST_RESULT
    from concourse import bass_utils

    wd1_np, wr1_np = _pack_blocks((w1f * A).astype(np.float32))
    wd2_np, wr2_np = _pack_blocks((w2f * A).astype(np.float32))
    wlin_np = np.zeros((128, N_TILES), dtype=bf16)
    for t in range(N_TILES):
        wlin_np[: P[t], t] = lw[OFF[t]:OFF[t] + P[t]].astype(bf16)

    nc = _build_program(b1f, b2f, linbf)

    # host-side: transpose, cast, shard along batch
    xt = np.ascontiguousarray(x.T).astype(bf16)        # [676, 65536]
    in_maps = []
    for c in range(N_CORES):
        in_maps.append({
            "xt": np.ascontiguousarray(xt[:, c * COLS:(c + 1) * COLS]),
            "wd1": wd1_np,
            "wd2": wd2_np,
            "wr1": wr1_np,
            "wr2": wr2_np,
            "wlin": wlin_np,
        })

    res = bass_utils.run_bass_kernel_spmd(
        nc, in_maps, list(range(N_CORES)), trace=TRACE
    )
    if TRACE:
        LAST_RESULT = res
    out = np.concatenate([res.results[c]["y"].reshape(-1) for c in range(N_CORES)])
    return out.reshape(B_TOTAL, 1).astype(np.float32)


def kernel(x, w1, b1, w2, b2, lin_w, lin_b, edge_src, edge_dst):
    import os

    x = np.asarray(x)
    # Build the dense normalized aggregation operator from the edge lists.
    deg = np.zeros(N, np.float64)
    np.add.at(deg, np.asarray(edge_dst), 1.0)
    dinv = 1.0 / np.sqrt(deg)
    normv = dinv[np.asarray(edge_src)] * dinv[np.asarray(edge_dst)]
    A = np.zeros((N, N), np.float64)
    np.add.at(A, (np.asarray(edge_dst), np.asarray(edge_src)), normv)

    w1f = float(np.asarray(w1).reshape(-1)[0])
    w2f = float(np.asarray(w2).reshape(-1)[0])
    b1f = float(np.asarray(b1).reshape(-1)[0])
    b2f = float(np.asarray(b2).reshape(-1)[0])
    linbf = float(np.asarray(lin_b).reshape(-1)[0])
    lw = np.asarray(lin_w).reshape(-1)

    # fp8 is bit-exact when w2 <= 0 and b2 <= 0 (h2 == 0 identically);
    # otherwise fall back to the bf16 kernel (~1e-3).
    use_fp8 = (w2f <= 0.0 and b2f <= 0.0) or os.environ.get("KERNEL_FORCE_FP8")
    if use_fp8:
        return _kernel_fp8(x, A, w1f, b1f, w2f, b2f, lw, linbf)
    return _kernel_bf16(x, A, w1f, b1f, w2f, b2f, lw, linbf)
